# revision 7
# baseline (speedup 1.0000x reference)
"""Trainium2 Bass kernel for NeuralMemoryODE.

Computes, for full inputs (B=8192, D=1024, H=2048, C=1000):
    gamma = x @ W_enc + b_enc
    y     = RK4(9 steps, dt=1/9) of dy/dt = -y + (1+exp(-y))*sin(y+gamma)^2
    out   = y @ W_cls + b_cls

Strategy: pure data-parallel over 8 NeuronCores (1024 batch rows each).
On-device layout is transposed ([H, B_core]) so biases are per-partition.
RK4 stage values are built on the TensorEngine as float32r scaled-identity
matmuls accumulating in PSUM; ScalarE evaluates sin/exp (sin args wrapped
into its valid domain once per step); VectorE does squares and the
(1+e)*q products via fused scalar_tensor_tensor ops.
"""

import sys
import os

if "/opt/trn_rl_repo" not in sys.path:
    sys.path.insert(0, "/opt/trn_rl_repo")

import numpy as np

import concourse.bacc as bacc
import concourse.mybir as mybir
import concourse.tile as tile
from concourse.tile import add_dep_helper
from concourse.bass_utils import run_bass_kernel_spmd

F32 = mybir.dt.float32
F32R = mybir.dt.float32r
BF16 = mybir.dt.bfloat16
AFT = mybir.ActivationFunctionType
ALU = mybir.AluOpType

P = 128
CB = 512                      # chunk free-dim width (one PSUM bank)
N_STEPS = 9
DT = 1.0 / N_STEPS
A = DT / 2.0
TWO_PI = 2.0 * np.pi
RC = 1.5 * 2.0**23            # round-to-nearest-even magic constant

# RK4 expansion coefficients (stage values as linear combos of y, g1..g4, U1w)
A1 = 1.0 - A
A2 = 1.0 - A + A * A
A3 = 1.0 - DT * A2
C0 = 1.0 - (DT / 6.0) * (1.0 + 2.0 * A1 + 2.0 * A2 + A3)
C1 = (DT / 6.0) * (1.0 - 2.0 * A + 2.0 * A * A - DT * A * A)
C2 = (DT / 6.0) * (2.0 - 2.0 * A + DT * A)
C3 = (DT / 6.0) * (2.0 - DT)
C4 = DT / 6.0

# identity coefficients, indexed by name
IDC = {
    "one": 1.0,
    "a": A, "na": -A,
    "A1": A1, "naA1": -A * A1, "naa": -A * A,
    "A2": A2,
    "dt": DT, "ndtA2": -DT * A2, "dtaa": DT * A * A, "ndta": -DT * A,
    "A3": A3,
    "c0": C0, "c1": C1, "c2": C2, "c3": C3, "c4": C4,
}
ID_NAMES = list(IDC.keys())
ID_IDX = {n: i for i, n in enumerate(ID_NAMES)}
NID = len(ID_NAMES)

# stage-value recipes: list of (ident_name, source) where source is one of
# "y", "g1".."g4", "U1w"
U2_R = [("one", "U1w"), ("a", "g1"), ("na", "y")]
Y2_R = [("A1", "y"), ("a", "g1")]
U3_R = [("one", "U1w"), ("a", "g2"), ("naA1", "y"), ("naa", "g1")]
Y3_R = [("A2", "y"), ("naa", "g1"), ("a", "g2")]
U4_R = [("one", "U1w"), ("dt", "g3"), ("ndtA2", "y"), ("dtaa", "g1"), ("ndta", "g2")]
Y4_R = [("A3", "y"), ("dtaa", "g1"), ("ndta", "g2"), ("dt", "g3")]
YN_R = [("c0", "y"), ("c1", "g1"), ("c2", "g2"), ("c3", "g3"), ("c4", "g4")]


def host_identities() -> np.ndarray:
    out = np.zeros((NID * P, P), dtype=np.float32)
    eye = np.eye(P, dtype=np.float32)
    for i, n in enumerate(ID_NAMES):
        out[i * P:(i + 1) * P, :] = np.float32(IDC[n]) * eye
    return out


def build_nc(H=2048, BC=1024, D=1024, CPAD=1024, n_steps=N_STEPS, G=11):
    """Build the per-core Bass program (same on all cores)."""
    HT = H // P
    KD = D // P
    NB = BC // CB
    KC = H // P           # classifier contraction tiles
    CT = CPAD // P        # classifier output row tiles
    n_chunks = HT * NB

    nc = bacc.Bacc("TRN2", target_bir_lowering=False, debug=False, num_devices=8)

    d_xT = nc.dram_tensor("xT", [D, BC], F32R, kind="ExternalInput")
    d_wenc = nc.dram_tensor("W_enc", [D, H], F32R, kind="ExternalInput")
    d_benc = nc.dram_tensor("b_enc", [H, 1], F32, kind="ExternalInput")
    d_wcls = nc.dram_tensor("W_cls", [H, CPAD], F32R, kind="ExternalInput")
    d_bcls = nc.dram_tensor("b_cls", [CPAD, 1], F32, kind="ExternalInput")
    d_ident = nc.dram_tensor("ident", [NID * P, P], F32R, kind="ExternalInput")
    d_zero = nc.dram_tensor("zeros", [P, CB], F32R, kind="ExternalInput")
    d_identb = nc.dram_tensor("identb", [NID * P, P], BF16, kind="ExternalInput")
    d_out = nc.dram_tensor("outT", [CPAD, BC], F32, kind="ExternalOutput")

    act_prev = [None]

    def act(*args, **kw):
        inst = nc.scalar.activation(*args, **kw).ins
        if act_prev[0] is not None:
            add_dep_helper(inst, act_prev[0], sync=False, reason="act-order")
        act_prev[0] = inst
        return inst

    with tile.TileContext(nc) as tc:
        with tc.tile_pool(name="dram", bufs=1, space="DRAM") as dpool:
            d_gam = dpool.tile([H, BC], F32R, name="gam_stage")
            d_yend = dpool.tile([H, BC], F32R, name="yend_stage")

            with tc.tile_pool(name="const", bufs=1) as cpool:
                idn = cpool.tile([P, NID * P], F32R, name="idn")
                for i in range(NID):
                    nc.sync.dma_start(idn[:, i * P:(i + 1) * P],
                                      d_ident.ap()[i * P:(i + 1) * P, :])

                idnb = cpool.tile([P, NID * P], BF16, name="idnb")
                for i in range(NID):
                    nc.sync.dma_start(idnb[:, i * P:(i + 1) * P],
                                      d_identb.ap()[i * P:(i + 1) * P, :])

                def ID(name):
                    i = ID_IDX[name]
                    return idn[:, i * P:(i + 1) * P]

                def IDB(name):
                    i = ID_IDX[name]
                    return idnb[:, i * P:(i + 1) * P]

                # ---------------- Phase E: encoder ----------------
                with tc.tile_pool(name="enc", bufs=1) as epool, \
                     tc.tile_pool(name="etmp", bufs=4) as etmp, \
                     tc.tile_pool(name="psum_e", bufs=8, space="PSUM") as epsum:
                    wenc_sb = []
                    for k in range(KD):
                        t = epool.tile([P, H], F32R, name=f"wenc{k}")
                        nc.sync.dma_start(t[:], d_wenc.ap()[k * P:(k + 1) * P, :])
                        wenc_sb.append(t)
                    xT_sb = []
                    for k in range(KD):
                        t = epool.tile([P, BC], F32R, name=f"xT{k}")
                        nc.sync.dma_start(t[:], d_xT.ap()[k * P:(k + 1) * P, :])
                        xT_sb.append(t)
                    benc_sb = epool.tile([P, HT], F32, name="benc")
                    nc.sync.dma_start(
                        benc_sb[:], d_benc.ap().rearrange("(t p) o -> p (t o)", p=P))

                    for ht in range(HT):
                        for nb in range(NB):
                            pg = epsum.tile([P, CB], F32, tag="pge")
                            for k in range(KD):
                                nc.tensor.matmul(
                                    pg[:], wenc_sb[k][:, ht * P:(ht + 1) * P],
                                    xT_sb[k][:, nb * CB:(nb + 1) * CB],
                                    start=(k == 0), stop=(k == KD - 1))
                            gf = etmp.tile([P, CB], F32, tag="gf")
                            act(gf[:], pg[:], AFT.Identity,
                                bias=benc_sb[:, ht:ht + 1])
                            gr = etmp.tile([P, CB], F32R, tag="gr")
                            nc.vector.tensor_scalar(gr[:], gf[:], 1.0, None, ALU.mult)
                            nc.sync.dma_start(
                                d_gam[ht * P:(ht + 1) * P, nb * CB:(nb + 1) * CB],
                                gr[:])

                # ---------------- Phase O: ODE ----------------
                chunks = [(ht, nb) for ht in range(HT) for nb in range(NB)]
                groups = [chunks[i:i + G] for i in range(0, n_chunks, G)]

                for gi, grp in enumerate(groups):
                    with tc.tile_pool(name=f"ode{gi}", bufs=1) as opool, \
                         tc.tile_pool(name=f"otmp{gi}", bufs=1) as otmp, \
                         tc.tile_pool(name=f"psum_o{gi}", bufs=8,
                                      space="PSUM") as opsum:
                        st = {}
                        for ci, (ht, nb) in enumerate(grp):
                            s = {}
                            s["gc"] = opool.tile([P, CB], F32R, name=f"gc{gi}_{ci}")
                            nc.sync.dma_start(
                                s["gc"][:],
                                d_gam[ht * P:(ht + 1) * P, nb * CB:(nb + 1) * CB])
                            s["yA"] = opool.tile([P, CB], F32R, name=f"yA{gi}_{ci}")
                            s["yB"] = opool.tile([P, CB], F32R, name=f"yB{gi}_{ci}")
                            nc.sync.dma_start(s["yA"][:], d_zero.ap())
                            s["U1w"] = opool.tile([P, CB], F32R, name=f"uw{gi}_{ci}")
                            for gn in ("g1", "g2", "g3", "g4"):
                                s[gn] = opool.tile([P, CB], BF16,
                                                   name=f"{gn}_{gi}_{ci}")
                            s["s"] = otmp.tile([P, CB], BF16, name=f"s{gi}_{ci}")
                            s["q"] = otmp.tile([P, CB], BF16, name=f"q{gi}_{ci}")
                            s["e"] = otmp.tile([P, CB], BF16, name=f"e{gi}_{ci}")
                            st[ci] = s

                        ncg = len(grp)

                        def mm_combo(dst_psum, recipe, srcs):
                            n = len(recipe)
                            for t, (idname, sname) in enumerate(recipe):
                                lhsT = IDB(idname) if sname.startswith("g") \
                                    else ID(idname)
                                nc.tensor.matmul(
                                    dst_psum[:], lhsT, srcs[sname],
                                    start=(t == 0), stop=(t == n - 1))

                        for step in range(n_steps):
                            cur, nxt = ("yA", "yB") if step % 2 == 0 else ("yB", "yA")
                            pu1, pY, pU, pYn = {}, {}, {}, {}

                            # u1 = y + gamma -> psum; wrap to U1w (sbuf, f32r)
                            for ci in range(ncg):
                                s = st[ci]
                                pu1[ci] = opsum.tile([P, CB], F32, tag="pp", name=f"pu1_{ci}")
                                nc.tensor.matmul(pu1[ci][:], ID("one"), s["gc"][:],
                                                 start=True, stop=False)
                                nc.tensor.matmul(pu1[ci][:], ID("one"), s[cur][:],
                                                 start=False, stop=True)
                            for ci in range(ncg):
                                s = st[ci]
                                m = otmp.tile([P, CB], F32, tag="wm", bufs=3,
                                              name=f"wm{ci}")
                                nc.vector.tensor_scalar(
                                    m[:], pu1[ci][:], 1.0 / TWO_PI, RC,
                                    ALU.mult, ALU.add)
                                n_t = otmp.tile([P, CB], F32, tag="wn", bufs=3,
                                                name=f"wn{ci}")
                                nc.vector.tensor_scalar(
                                    n_t[:], m[:], RC, None, ALU.subtract)
                                nc.vector.scalar_tensor_tensor(
                                    s["U1w"][:], n_t[:], -TWO_PI, pu1[ci][:],
                                    ALU.mult, ALU.add)

                            for stg in range(4):
                                gname = f"g{stg + 1}"
                                # stage inputs: psum Y_i (exp) and U_i (sin)
                                if stg == 0:
                                    for ci in range(ncg):
                                        act(st[ci]["e"][:],
                                            st[ci][cur][:].bitcast(F32),
                                            AFT.Exp, scale=-1.0)
                                    for ci in range(ncg):
                                        act(st[ci]["s"][:],
                                            st[ci]["U1w"][:].bitcast(F32),
                                            AFT.Sin)
                                else:
                                    yr, ur = [(Y2_R, U2_R), (Y3_R, U3_R),
                                              (Y4_R, U4_R)][stg - 1]
                                    for ci in range(ncg):
                                        s = st[ci]
                                        srcs = {"y": s[cur][:], "U1w": s["U1w"][:],
                                                "g1": s["g1"][:], "g2": s["g2"][:],
                                                "g3": s["g3"][:], "g4": s["g4"][:]}
                                        pY[ci] = opsum.tile([P, CB], F32, tag="pp", name=f"pY_{ci}")
                                        mm_combo(pY[ci], yr, srcs)
                                    for ci in range(ncg):
                                        act(st[ci]["e"][:], pY[ci][:],
                                            AFT.Exp, scale=-1.0)
                                    for ci in range(ncg):
                                        s = st[ci]
                                        srcs = {"y": s[cur][:], "U1w": s["U1w"][:],
                                                "g1": s["g1"][:], "g2": s["g2"][:],
                                                "g3": s["g3"][:], "g4": s["g4"][:]}
                                        pU[ci] = opsum.tile([P, CB], F32, tag="pp", name=f"pU_{ci}")
                                        mm_combo(pU[ci], ur, srcs)
                                    for ci in range(ncg):
                                        act(st[ci]["s"][:], pU[ci][:], AFT.Sin)
                                for ci in range(ncg):
                                    s = st[ci]
                                    nc.vector.tensor_mul(s["q"][:], s["s"][:],
                                                         s["s"][:])
                                for ci in range(ncg):
                                    s = st[ci]
                                    nc.vector.scalar_tensor_tensor(
                                        s[gname][:], s["e"][:], 1.0, s["q"][:],
                                        ALU.add, ALU.mult)

                            # y_next = C0*y + sum Ci*gi -> psum -> sbuf
                            for ci in range(ncg):
                                s = st[ci]
                                srcs = {"y": s[cur][:], "U1w": s["U1w"][:],
                                        "g1": s["g1"][:], "g2": s["g2"][:],
                                        "g3": s["g3"][:], "g4": s["g4"][:]}
                                pYn[ci] = opsum.tile([P, CB], F32, tag="pp", name=f"pYn_{ci}")
                                mm_combo(pYn[ci], YN_R, srcs)
                            for ci in range(ncg):
                                nc.vector.tensor_copy(st[ci][nxt][:], pYn[ci][:])

                        fin = "yA" if n_steps % 2 == 0 else "yB"
                        for ci, (ht, nb) in enumerate(grp):
                            nc.sync.dma_start(
                                d_yend[ht * P:(ht + 1) * P,
                                       nb * CB:(nb + 1) * CB],
                                st[ci][fin][:])

                # ---------------- Phase C: classifier ----------------
                with tc.tile_pool(name="cls", bufs=1) as clpool, \
                     tc.tile_pool(name="ctmp", bufs=4) as ctmp, \
                     tc.tile_pool(name="cstr", bufs=2 * KC) as cstr, \
                     tc.tile_pool(name="psum_c", bufs=8, space="PSUM") as cpsum:
                    wcls_sb = []
                    for k in range(KC):
                        t = clpool.tile([P, CPAD], F32R, name=f"wcls{k}")
                        nc.sync.dma_start(t[:], d_wcls.ap()[k * P:(k + 1) * P, :])
                        wcls_sb.append(t)
                    bcls_sb = clpool.tile([P, CT], F32, name="bcls")
                    nc.sync.dma_start(
                        bcls_sb[:], d_bcls.ap().rearrange("(t p) o -> p (t o)", p=P))

                    for nb in range(NB):
                        ye_sb = []
                        for k in range(KC):
                            t = cstr.tile([P, CB], F32R, tag="yend_t")
                            nc.sync.dma_start(
                                t[:], d_yend[k * P:(k + 1) * P,
                                             nb * CB:(nb + 1) * CB])
                            ye_sb.append(t)
                        for ct in range(CT):
                            pc = cpsum.tile([P, CB], F32, tag="pcl")
                            for k in range(KC):
                                nc.tensor.matmul(
                                    pc[:], wcls_sb[k][:, ct * P:(ct + 1) * P],
                                    ye_sb[k][:], start=(k == 0),
                                    stop=(k == KC - 1))
                            ot = ctmp.tile([P, CB], F32, tag="ot")
                            act(ot[:], pc[:], AFT.Identity,
                                bias=bcls_sb[:, ct:ct + 1])
                            nc.sync.dma_start(
                                d_out.ap()[ct * P:(ct + 1) * P,
                                           nb * CB:(nb + 1) * CB], ot[:])

    nc.compile()
    return nc


_cached = {}


def _get_nc(key):
    if key not in _cached:
        H, BC, D, CPAD, n_steps, G = key
        _cached[key] = build_nc(H=H, BC=BC, D=D, CPAD=CPAD, n_steps=n_steps, G=G)
    return _cached[key]


def _prepare(x, W_enc, b_enc, W_cls, b_cls, G=11):
    B, D = x.shape
    H = W_enc.shape[1]
    C = W_cls.shape[1]
    NCORES = 8
    BC = B // NCORES
    CPAD = ((C + P - 1) // P) * P

    nc = _get_nc((H, BC, D, CPAD, N_STEPS, G))

    wcls_pad = np.zeros((H, CPAD), dtype=np.float32)
    wcls_pad[:, :C] = W_cls
    bcls_pad = np.zeros((CPAD, 1), dtype=np.float32)
    bcls_pad[:C, 0] = b_cls
    ident = host_identities()
    import ml_dtypes
    identb = ident.astype(ml_dtypes.bfloat16)
    benc = np.ascontiguousarray(b_enc.reshape(H, 1).astype(np.float32))
    wenc = np.ascontiguousarray(W_enc.astype(np.float32))

    in_maps = []
    for c in range(NCORES):
        xT = np.ascontiguousarray(x[c * BC:(c + 1) * BC, :].T.astype(np.float32))
        in_maps.append({
            "xT": xT, "W_enc": wenc, "b_enc": benc,
            "W_cls": wcls_pad, "b_cls": bcls_pad, "ident": ident,
            "identb": identb,
            "zeros": np.zeros((P, CB), dtype=np.float32),
        })
    return nc, in_maps, (B, C, BC, NCORES)


def _gather(res, shape):
    B, C, BC, NCORES = shape
    out = np.empty((B, C), dtype=np.float32)
    for c in range(NCORES):
        out[c * BC:(c + 1) * BC, :] = res.results[c]["outT"][:C, :].T
    return out


def kernel(x, W_enc, b_enc, W_cls, b_cls):
    nc, in_maps, shape = _prepare(x, W_enc, b_enc, W_cls, b_cls)
    res = run_bass_kernel_spmd(nc, in_maps, list(range(shape[3])))
    return _gather(res, shape)


def kernel_traced(x, W_enc, b_enc, W_cls, b_cls, G=11, **trace_kw):
    nc, in_maps, shape = _prepare(x, W_enc, b_enc, W_cls, b_cls, G=G)
    res = run_bass_kernel_spmd(nc, in_maps, list(range(shape[3])),
                               trace=True, **trace_kw)
    return _gather(res, shape), res


# revision 9
# speedup vs baseline: 1.0138x; 1.0138x over previous
"""Trainium2 Bass kernel for NeuralMemoryODE.

Computes, for full inputs (B=8192, D=1024, H=2048, C=1000):
    gamma = x @ W_enc + b_enc
    y     = RK4(9 steps, dt=1/9) of dy/dt = -y + (1+exp(-y))*sin(y+gamma)^2
    out   = y @ W_cls + b_cls

Strategy: pure data-parallel over 8 NeuronCores (1024 batch rows each).
On-device layout is transposed ([H, B_core]) so biases are per-partition.
RK4 stage values are built on the TensorEngine as float32r scaled-identity
matmuls accumulating in PSUM; ScalarE evaluates sin/exp (sin args wrapped
into its valid domain once per step); VectorE does squares and the
(1+e)*q products via fused scalar_tensor_tensor ops.
"""

import sys
import os

if "/opt/trn_rl_repo" not in sys.path:
    sys.path.insert(0, "/opt/trn_rl_repo")

import numpy as np

import concourse.bacc as bacc
import concourse.mybir as mybir
import concourse.tile as tile
from concourse.tile import add_dep_helper
from concourse.bass_utils import run_bass_kernel_spmd

F32 = mybir.dt.float32
ACT_CHAIN = True
PSUM_BUFS = 8
F32R = mybir.dt.float32r
BF16 = mybir.dt.bfloat16
AFT = mybir.ActivationFunctionType
ALU = mybir.AluOpType

P = 128
CB = 512                      # chunk free-dim width (one PSUM bank)
N_STEPS = 9
DT = 1.0 / N_STEPS
A = DT / 2.0
TWO_PI = 2.0 * np.pi
RC = 1.5 * 2.0**23            # round-to-nearest-even magic constant

# RK4 expansion coefficients (stage values as linear combos of y, g1..g4, U1w)
A1 = 1.0 - A
A2 = 1.0 - A + A * A
A3 = 1.0 - DT * A2
C0 = 1.0 - (DT / 6.0) * (1.0 + 2.0 * A1 + 2.0 * A2 + A3)
C1 = (DT / 6.0) * (1.0 - 2.0 * A + 2.0 * A * A - DT * A * A)
C2 = (DT / 6.0) * (2.0 - 2.0 * A + DT * A)
C3 = (DT / 6.0) * (2.0 - DT)
C4 = DT / 6.0

# identity coefficients, indexed by name
IDC = {
    "one": 1.0,
    "a": A, "na": -A,
    "A1": A1, "naA1": -A * A1, "naa": -A * A,
    "A2": A2,
    "dt": DT, "ndtA2": -DT * A2, "dtaa": DT * A * A, "ndta": -DT * A,
    "A3": A3,
    "c0": C0, "c1": C1, "c2": C2, "c3": C3, "c4": C4,
}
ID_NAMES = list(IDC.keys())
ID_IDX = {n: i for i, n in enumerate(ID_NAMES)}
NID = len(ID_NAMES)

# stage-value recipes: list of (ident_name, source) where source is one of
# "y", "g1".."g4", "U1w"
U2_R = [("one", "U1w"), ("a", "g1"), ("na", "y")]
Y2_R = [("A1", "y"), ("a", "g1")]
U3_R = [("one", "U1w"), ("a", "g2"), ("naA1", "y"), ("naa", "g1")]
Y3_R = [("A2", "y"), ("naa", "g1"), ("a", "g2")]
U4_R = [("one", "U1w"), ("dt", "g3"), ("ndtA2", "y"), ("dtaa", "g1"), ("ndta", "g2")]
Y4_R = [("A3", "y"), ("dtaa", "g1"), ("ndta", "g2"), ("dt", "g3")]
YN_R = [("c0", "y"), ("c1", "g1"), ("c2", "g2"), ("c3", "g3"), ("c4", "g4")]

# step-0 variants (y=0: all y-terms vanish)
U2_R0 = [("one", "U1w"), ("a", "g1")]
Y2_R0 = [("a", "g1")]
U3_R0 = [("one", "U1w"), ("a", "g2"), ("naa", "g1")]
Y3_R0 = [("naa", "g1"), ("a", "g2")]
U4_R0 = [("one", "U1w"), ("dt", "g3"), ("dtaa", "g1"), ("ndta", "g2")]
Y4_R0 = [("dtaa", "g1"), ("ndta", "g2"), ("dt", "g3")]
YN_R0 = [("c1", "g1"), ("c2", "g2"), ("c3", "g3"), ("c4", "g4")]


def host_identities() -> np.ndarray:
    out = np.zeros((NID * P, P), dtype=np.float32)
    eye = np.eye(P, dtype=np.float32)
    for i, n in enumerate(ID_NAMES):
        out[i * P:(i + 1) * P, :] = np.float32(IDC[n]) * eye
    return out


def build_nc(H=2048, BC=1024, D=1024, CPAD=1024, n_steps=N_STEPS, G=11):
    """Build the per-core Bass program (same on all cores)."""
    HT = H // P
    KD = D // P
    NB = BC // CB
    KC = H // P           # classifier contraction tiles
    CT = CPAD // P        # classifier output row tiles
    n_chunks = HT * NB

    nc = bacc.Bacc("TRN2", target_bir_lowering=False, debug=False, num_devices=8)

    d_xT = nc.dram_tensor("xT", [D, BC], F32R, kind="ExternalInput")
    d_wenc = nc.dram_tensor("W_enc", [D, H], F32R, kind="ExternalInput")
    d_benc = nc.dram_tensor("b_enc", [H, 1], F32, kind="ExternalInput")
    d_wcls = nc.dram_tensor("W_cls", [H, CPAD], F32R, kind="ExternalInput")
    d_bcls = nc.dram_tensor("b_cls", [CPAD, 1], F32, kind="ExternalInput")
    d_ident = nc.dram_tensor("ident", [NID * P, P], F32R, kind="ExternalInput")
    d_zero = nc.dram_tensor("zeros", [P, CB], F32R, kind="ExternalInput")
    d_identb = nc.dram_tensor("identb", [NID * P, P], BF16, kind="ExternalInput")
    d_out = nc.dram_tensor("outT", [CPAD, BC], F32, kind="ExternalOutput")

    act_prev = [None]

    def act(*args, **kw):
        inst = nc.scalar.activation(*args, **kw).ins
        if ACT_CHAIN and act_prev[0] is not None:
            add_dep_helper(inst, act_prev[0], sync=False, reason="act-order")
        act_prev[0] = inst
        return inst

    with tile.TileContext(nc) as tc:
        with tc.tile_pool(name="dram", bufs=1, space="DRAM") as dpool:
            d_gam = dpool.tile([H, BC], F32R, name="gam_stage")
            d_yend = dpool.tile([H, BC], F32R, name="yend_stage")

            with tc.tile_pool(name="const", bufs=1) as cpool:
                idn = cpool.tile([P, NID * P], F32R, name="idn")
                for i in range(NID):
                    nc.sync.dma_start(idn[:, i * P:(i + 1) * P],
                                      d_ident.ap()[i * P:(i + 1) * P, :])

                idnb = cpool.tile([P, NID * P], BF16, name="idnb")
                for i in range(NID):
                    nc.sync.dma_start(idnb[:, i * P:(i + 1) * P],
                                      d_identb.ap()[i * P:(i + 1) * P, :])

                def ID(name):
                    i = ID_IDX[name]
                    return idn[:, i * P:(i + 1) * P]

                def IDB(name):
                    i = ID_IDX[name]
                    return idnb[:, i * P:(i + 1) * P]

                # ---------------- Phase E: encoder ----------------
                with tc.tile_pool(name="enc", bufs=1) as epool, \
                     tc.tile_pool(name="etmp", bufs=4) as etmp, \
                     tc.tile_pool(name="psum_e", bufs=8, space="PSUM") as epsum:
                    wenc_sb = []
                    for k in range(KD):
                        t = epool.tile([P, H], F32R, name=f"wenc{k}")
                        nc.sync.dma_start(t[:], d_wenc.ap()[k * P:(k + 1) * P, :])
                        wenc_sb.append(t)
                    xT_sb = []
                    for k in range(KD):
                        t = epool.tile([P, BC], F32R, name=f"xT{k}")
                        nc.sync.dma_start(t[:], d_xT.ap()[k * P:(k + 1) * P, :])
                        xT_sb.append(t)
                    benc_sb = epool.tile([P, HT], F32, name="benc")
                    nc.sync.dma_start(
                        benc_sb[:], d_benc.ap().rearrange("(t p) o -> p (t o)", p=P))

                    for ht in range(HT):
                        for nb in range(NB):
                            pg = epsum.tile([P, CB], F32, tag="pge")
                            for k in range(KD):
                                nc.tensor.matmul(
                                    pg[:], wenc_sb[k][:, ht * P:(ht + 1) * P],
                                    xT_sb[k][:, nb * CB:(nb + 1) * CB],
                                    start=(k == 0), stop=(k == KD - 1))
                            gf = etmp.tile([P, CB], F32, tag="gf")
                            act(gf[:], pg[:], AFT.Identity,
                                bias=benc_sb[:, ht:ht + 1])
                            gr = etmp.tile([P, CB], F32R, tag="gr")
                            nc.vector.tensor_scalar(gr[:], gf[:], 1.0, None, ALU.mult)
                            nc.sync.dma_start(
                                d_gam[ht * P:(ht + 1) * P, nb * CB:(nb + 1) * CB],
                                gr[:])

                # ---------------- Phase O: ODE ----------------
                chunks = [(ht, nb) for ht in range(HT) for nb in range(NB)]
                groups = [chunks[i:i + G] for i in range(0, n_chunks, G)]

                for gi, grp in enumerate(groups):
                    with tc.tile_pool(name=f"ode{gi}", bufs=1) as opool, \
                         tc.tile_pool(name=f"otmp{gi}", bufs=1) as otmp, \
                         tc.tile_pool(name=f"psum_o{gi}", bufs=PSUM_BUFS,
                                      space="PSUM") as opsum:
                        st = {}
                        for ci, (ht, nb) in enumerate(grp):
                            s = {}
                            s["gc"] = opool.tile([P, CB], F32R, name=f"gc{gi}_{ci}")
                            nc.sync.dma_start(
                                s["gc"][:],
                                d_gam[ht * P:(ht + 1) * P, nb * CB:(nb + 1) * CB])
                            s["yA"] = opool.tile([P, CB], F32R, name=f"yA{gi}_{ci}")
                            s["yB"] = opool.tile([P, CB], F32R, name=f"yB{gi}_{ci}")
                            s["U1w"] = opool.tile([P, CB], F32R, name=f"uw{gi}_{ci}")
                            for gn in ("g1", "g2", "g3", "g4"):
                                s[gn] = opool.tile([P, CB], BF16,
                                                   name=f"{gn}_{gi}_{ci}")
                            s["s"] = otmp.tile([P, CB], BF16, name=f"s{gi}_{ci}")
                            s["q"] = otmp.tile([P, CB], BF16, name=f"q{gi}_{ci}")
                            s["e"] = otmp.tile([P, CB], BF16, name=f"e{gi}_{ci}")
                            st[ci] = s

                        ncg = len(grp)

                        def mm_combo(dst_psum, recipe, srcs):
                            n = len(recipe)
                            for t, (idname, sname) in enumerate(recipe):
                                lhsT = IDB(idname) if sname.startswith("g") \
                                    else ID(idname)
                                nc.tensor.matmul(
                                    dst_psum[:], lhsT, srcs[sname],
                                    start=(t == 0), stop=(t == n - 1))

                        for step in range(n_steps):
                            cur, nxt = ("yA", "yB") if step % 2 == 0 else ("yB", "yA")
                            pu1, pY, pU, pYn = {}, {}, {}, {}

                            # u1 = y + gamma -> psum; wrap to U1w (sbuf, f32r)
                            if step > 0:
                                for ci in range(ncg):
                                    s = st[ci]
                                    pu1[ci] = opsum.tile([P, CB], F32, tag="pp", name=f"pu1_{ci}")
                                    nc.tensor.matmul(pu1[ci][:], ID("one"), s["gc"][:],
                                                     start=True, stop=False)
                                    nc.tensor.matmul(pu1[ci][:], ID("one"), s[cur][:],
                                                     start=False, stop=True)
                            for ci in range(ncg):
                                s = st[ci]
                                u1src = (s["gc"][:].bitcast(F32) if step == 0
                                         else pu1[ci][:])
                                m = otmp.tile([P, CB], F32, tag="wm", bufs=3,
                                              name=f"wm{ci}")
                                nc.vector.tensor_scalar(
                                    m[:], u1src, 1.0 / TWO_PI, RC,
                                    ALU.mult, ALU.add)
                                n_t = otmp.tile([P, CB], F32, tag="wn", bufs=3,
                                                name=f"wn{ci}")
                                nc.vector.tensor_scalar(
                                    n_t[:], m[:], RC, None, ALU.subtract)
                                nc.vector.scalar_tensor_tensor(
                                    s["U1w"][:], n_t[:], -TWO_PI, u1src,
                                    ALU.mult, ALU.add)

                            for stg in range(4):
                                gname = f"g{stg + 1}"
                                # stage inputs: psum Y_i (exp) and U_i (sin)
                                if stg == 0:
                                    if step > 0:
                                        for ci in range(ncg):
                                            act(st[ci]["e"][:],
                                                st[ci][cur][:].bitcast(F32),
                                                AFT.Exp, scale=-1.0)
                                    for ci in range(ncg):
                                        act(st[ci]["s"][:],
                                            st[ci]["U1w"][:].bitcast(F32),
                                            AFT.Sin)
                                else:
                                    if step == 0:
                                        yr, ur = [(Y2_R0, U2_R0), (Y3_R0, U3_R0),
                                                  (Y4_R0, U4_R0)][stg - 1]
                                    else:
                                        yr, ur = [(Y2_R, U2_R), (Y3_R, U3_R),
                                                  (Y4_R, U4_R)][stg - 1]
                                    for ci in range(ncg):
                                        s = st[ci]
                                        srcs = {"y": s[cur][:], "U1w": s["U1w"][:],
                                                "g1": s["g1"][:], "g2": s["g2"][:],
                                                "g3": s["g3"][:], "g4": s["g4"][:]}
                                        pY[ci] = opsum.tile([P, CB], F32, tag="pp", name=f"pY_{ci}")
                                        mm_combo(pY[ci], yr, srcs)
                                    for ci in range(ncg):
                                        act(st[ci]["e"][:], pY[ci][:],
                                            AFT.Exp, scale=-1.0)
                                    for ci in range(ncg):
                                        s = st[ci]
                                        srcs = {"y": s[cur][:], "U1w": s["U1w"][:],
                                                "g1": s["g1"][:], "g2": s["g2"][:],
                                                "g3": s["g3"][:], "g4": s["g4"][:]}
                                        pU[ci] = opsum.tile([P, CB], F32, tag="pp", name=f"pU_{ci}")
                                        mm_combo(pU[ci], ur, srcs)
                                    for ci in range(ncg):
                                        act(st[ci]["s"][:], pU[ci][:], AFT.Sin)
                                for ci in range(ncg):
                                    s = st[ci]
                                    nc.vector.tensor_mul(s["q"][:], s["s"][:],
                                                         s["s"][:])
                                for ci in range(ncg):
                                    s = st[ci]
                                    if step == 0 and stg == 0:
                                        nc.vector.tensor_scalar(
                                            s[gname][:], s["q"][:], 2.0, None,
                                            ALU.mult)
                                    else:
                                        nc.vector.scalar_tensor_tensor(
                                            s[gname][:], s["e"][:], 1.0, s["q"][:],
                                            ALU.add, ALU.mult)

                            # y_next = C0*y + sum Ci*gi -> psum -> sbuf
                            for ci in range(ncg):
                                s = st[ci]
                                srcs = {"y": s[cur][:], "U1w": s["U1w"][:],
                                        "g1": s["g1"][:], "g2": s["g2"][:],
                                        "g3": s["g3"][:], "g4": s["g4"][:]}
                                pYn[ci] = opsum.tile([P, CB], F32, tag="pp", name=f"pYn_{ci}")
                                mm_combo(pYn[ci], YN_R0 if step == 0 else YN_R,
                                         srcs)
                            for ci in range(ncg):
                                nc.vector.tensor_copy(st[ci][nxt][:], pYn[ci][:])

                        fin = "yA" if n_steps % 2 == 0 else "yB"
                        for ci, (ht, nb) in enumerate(grp):
                            nc.sync.dma_start(
                                d_yend[ht * P:(ht + 1) * P,
                                       nb * CB:(nb + 1) * CB],
                                st[ci][fin][:])

                # ---------------- Phase C: classifier ----------------
                with tc.tile_pool(name="cls", bufs=1) as clpool, \
                     tc.tile_pool(name="ctmp", bufs=4) as ctmp, \
                     tc.tile_pool(name="cstr", bufs=2 * KC) as cstr, \
                     tc.tile_pool(name="psum_c", bufs=8, space="PSUM") as cpsum:
                    wcls_sb = []
                    for k in range(KC):
                        t = clpool.tile([P, CPAD], F32R, name=f"wcls{k}")
                        nc.sync.dma_start(t[:], d_wcls.ap()[k * P:(k + 1) * P, :])
                        wcls_sb.append(t)
                    bcls_sb = clpool.tile([P, CT], F32, name="bcls")
                    nc.sync.dma_start(
                        bcls_sb[:], d_bcls.ap().rearrange("(t p) o -> p (t o)", p=P))

                    for nb in range(NB):
                        ye_sb = []
                        for k in range(KC):
                            t = cstr.tile([P, CB], F32R, tag="yend_t")
                            nc.sync.dma_start(
                                t[:], d_yend[k * P:(k + 1) * P,
                                             nb * CB:(nb + 1) * CB])
                            ye_sb.append(t)
                        for ct in range(CT):
                            pc = cpsum.tile([P, CB], F32, tag="pcl")
                            for k in range(KC):
                                nc.tensor.matmul(
                                    pc[:], wcls_sb[k][:, ct * P:(ct + 1) * P],
                                    ye_sb[k][:], start=(k == 0),
                                    stop=(k == KC - 1))
                            ot = ctmp.tile([P, CB], F32, tag="ot")
                            act(ot[:], pc[:], AFT.Identity,
                                bias=bcls_sb[:, ct:ct + 1])
                            nc.sync.dma_start(
                                d_out.ap()[ct * P:(ct + 1) * P,
                                           nb * CB:(nb + 1) * CB], ot[:])

    nc.compile()
    return nc


_cached = {}


def _get_nc(key):
    if key not in _cached:
        H, BC, D, CPAD, n_steps, G = key
        _cached[key] = build_nc(H=H, BC=BC, D=D, CPAD=CPAD, n_steps=n_steps, G=G)
    return _cached[key]


def _prepare(x, W_enc, b_enc, W_cls, b_cls, G=11):
    B, D = x.shape
    H = W_enc.shape[1]
    C = W_cls.shape[1]
    NCORES = 8
    BC = B // NCORES
    CPAD = ((C + P - 1) // P) * P

    nc = _get_nc((H, BC, D, CPAD, N_STEPS, G))

    wcls_pad = np.zeros((H, CPAD), dtype=np.float32)
    wcls_pad[:, :C] = W_cls
    bcls_pad = np.zeros((CPAD, 1), dtype=np.float32)
    bcls_pad[:C, 0] = b_cls
    ident = host_identities()
    import ml_dtypes
    identb = ident.astype(ml_dtypes.bfloat16)
    benc = np.ascontiguousarray(b_enc.reshape(H, 1).astype(np.float32))
    wenc = np.ascontiguousarray(W_enc.astype(np.float32))

    in_maps = []
    for c in range(NCORES):
        xT = np.ascontiguousarray(x[c * BC:(c + 1) * BC, :].T.astype(np.float32))
        in_maps.append({
            "xT": xT, "W_enc": wenc, "b_enc": benc,
            "W_cls": wcls_pad, "b_cls": bcls_pad, "ident": ident,
            "identb": identb,
            "zeros": np.zeros((P, CB), dtype=np.float32),
        })
    return nc, in_maps, (B, C, BC, NCORES)


def _gather(res, shape):
    B, C, BC, NCORES = shape
    out = np.empty((B, C), dtype=np.float32)
    for c in range(NCORES):
        out[c * BC:(c + 1) * BC, :] = res.results[c]["outT"][:C, :].T
    return out


def kernel(x, W_enc, b_enc, W_cls, b_cls):
    nc, in_maps, shape = _prepare(x, W_enc, b_enc, W_cls, b_cls)
    res = run_bass_kernel_spmd(nc, in_maps, list(range(shape[3])))
    return _gather(res, shape)


def kernel_traced(x, W_enc, b_enc, W_cls, b_cls, G=11, **trace_kw):
    nc, in_maps, shape = _prepare(x, W_enc, b_enc, W_cls, b_cls, G=G)
    res = run_bass_kernel_spmd(nc, in_maps, list(range(shape[3])),
                               trace=True, **trace_kw)
    return _gather(res, shape), res
